# revision 6
# baseline (speedup 1.0000x reference)
"""AcaiCLIP FILIP-score + DCL-loss kernel for 8 Trainium2 NeuronCores.

Sharding: data-parallel over the bA axis (32 samples -> 4 per core). Each core
projects its hA slice, normalizes both sides (with 0/1 mask-zeroing folded into
the normalization scale), computes its [4,32,256,256] similarity slab with f32r
matmuls, reduces row-maxes (free-dim reduce) and col-maxes (DVE transpose-mode
reduce) straight from PSUM, and emits masked-mean numerators. Host assembles
the [32,32] logits and the scalar DCL loss.
"""
import os
import sys

for _p in ("/opt/trn_rl_repo", "/root/.axon_site/_ro/trn_rl_repo"):
    if os.path.isdir(_p) and _p not in sys.path:
        sys.path.insert(0, _p)

import numpy as np

import concourse.bacc as bacc
import concourse.mybir as mybir
import concourse.tile as tile
from concourse.bass_utils import run_bass_kernel_spmd
from concourse.masks import make_identity

F32 = mybir.dt.float32
F32R = mybir.dt.float32r
AX = mybir.AxisListType
OP = mybir.AluOpType
ACTF = mybir.ActivationFunctionType

B, TA, TB, DA, D = 32, 256, 256, 1024, 256
NCORES = 8
BA_PER = B // NCORES              # 4 a-samples per core
NTOK_A = BA_PER * TA              # 1024 hA tokens per core
NTOK_B = B * TB                   # 8192 hB tokens (full)
NPT_A = NTOK_A // 128             # 8 hA token tiles
NPT_B = NTOK_B // 128             # 64 hB token tiles
NKC = DA // 128                   # 8 contraction chunks for projection
GRP = 2048                        # tB columns per PSUM group (4 banks)
NGRP = NTOK_B // GRP              # 4 groups

_NC = None
LAST_RESULTS = None


def _round_to_f32r(nc, pool, src_tile, shape, name):
    t = pool.tile(shape, F32R, name=name, tag=name)
    nc.vector.tensor_copy(t[:], src_tile)
    return t


def build_program():
    nc = bacc.Bacc("TRN2")

    hat = nc.dram_tensor("hat", [NTOK_A, DA], F32, kind="ExternalInput")     # hA_raw^T slice: [dA, tok] stored [1024,1024]
    wt = nc.dram_tensor("wt", [DA, D], F32, kind="ExternalInput")            # W^T: [dA, d]
    brow = nc.dram_tensor("brow", [1, D], F32, kind="ExternalInput")
    hb = nc.dram_tensor("hb", [NTOK_B, D], F32, kind="ExternalInput")
    wa = nc.dram_tensor("wa", [128, NPT_A], F32, kind="ExternalInput")       # maskA 0/1, token t=pt*128+p -> [p, pt]
    wb = nc.dram_tensor("wb", [128, NPT_B], F32, kind="ExternalInput")       # maskB 0/1 same layout
    wbs = nc.dram_tensor("wbs", [32, 256], F32, kind="ExternalInput")        # maskB in shuffled colmax layout
    osums = nc.dram_tensor("osums", [32, 2 * BA_PER], F32, kind="ExternalOutput")

    with tile.TileContext(nc) as tc:
        with tc.tile_pool(name="const", bufs=1) as constp, \
             tc.tile_pool(name="persist", bufs=1) as pp, \
             tc.tile_pool(name="stage", bufs=1) as stp:

            # --- constants ---
            ident_f = constp.tile([128, 128], F32)
            make_identity(nc, ident_f[:])
            identr = _round_to_f32r(nc, constp, ident_f[:], [128, 128], 'identr')
            ones_f = constp.tile([128, 1], F32)
            nc.vector.memset(ones_f[:], 1.0)
            onesr = _round_to_f32r(nc, constp, ones_f[:], [128, 1], 'onesr')
            onesk_f = constp.tile([1, 128], F32)
            nc.vector.memset(onesk_f[:], 1.0)
            oneskr = _round_to_f32r(nc, constp, onesk_f[:], [1, 128], 'oneskr')

            # --- persistent data ---
            hatr = [pp.tile([128, NTOK_A], F32R, tag=f"hatr{k}", name=f"hatr{k}") for k in range(NKC)]
            wtr = [pp.tile([128, D], F32R, tag=f"wtr{k}", name=f"wtr{k}") for k in range(NKC)]
            browr = pp.tile([1, D], F32R)
            hAt = [pp.tile([128, NTOK_A], F32R, tag=f"hAt{c}", name=f"hAt{c}") for c in range(2)]
            hBt = [pp.tile([128, NTOK_B], F32R, tag=f"hBt{c}", name=f"hBt{c}") for c in range(2)]
            wat = stp.tile([128, NPT_A], F32)
            wbt = stp.tile([128, NPT_B], F32)
            wbst = stp.tile([32, 256], F32)
            nc.sync.dma_start(wat[:], wa[:])
            nc.sync.dma_start(wbt[:], wb[:])
            nc.sync.dma_start(wbst[:], wbs[:])

            # --- load + round projection operands ---
            with tc.tile_pool(name="ldst", bufs=3) as ldst:
                for k in range(NKC):
                    s = ldst.tile([128, NTOK_A], F32, tag="hatst")
                    nc.sync.dma_start(s[:], hat[k * 128:(k + 1) * 128, :])
                    nc.vector.tensor_copy(hatr[k][:], s[:])
                for k in range(NKC):
                    s = ldst.tile([128, D], F32, tag="wtst")
                    nc.sync.dma_start(s[:], wt[k * 128:(k + 1) * 128, :])
                    nc.vector.tensor_copy(wtr[k][:], s[:])
                s = ldst.tile([1, D], F32, tag="brst")
                nc.sync.dma_start(s[:], brow[:])
                nc.vector.tensor_copy(browr[:], s[:])

            # =========== Phase A: project hA, normalize+mask, transpose ===========
            ssA = stp.tile([128, NPT_A], F32)
            scaleA = stp.tile([128, NPT_A], F32)
            with tc.tile_pool(name="psA", bufs=4, space="PSUM") as psA, \
                 tc.tile_pool(name="psTA", bufs=2, space="PSUM") as psTA, \
                 tc.tile_pool(name="scrA", bufs=2) as scrA, \
                 tc.tile_pool(name="hAn", bufs=2) as hAnp:
                for wave in range(2):
                    ptiles = []
                    for pt in range(wave * 4, wave * 4 + 4):
                        ps = psA.tile([128, D], F32, tag="psA")
                        for k in range(NKC):
                            nc.tensor.matmul(
                                ps[:], hatr[k][:, pt * 128:(pt + 1) * 128], wtr[k][:],
                                start=(k == 0), stop=False)
                        nc.tensor.matmul(ps[:], oneskr[:], browr[:], start=False, stop=True)
                        scr = scrA.tile([128, D], F32, tag="scrA")
                        nc.scalar.activation(scr[:], ps[:], ACTF.Square,
                                             accum_out=ssA[:, pt:pt + 1])
                        ptiles.append((pt, ps))
                    # sqrt/recip/mask for this wave (batched over its 4 cols)
                    sl = slice(wave * 4, wave * 4 + 4)
                    nc.scalar.activation(scaleA[:, sl], ssA[:, sl], ACTF.Sqrt)
                    nc.vector.reciprocal(scaleA[:, sl], scaleA[:, sl])
                    nc.vector.tensor_tensor(scaleA[:, sl], scaleA[:, sl], wat[:, sl], op=OP.mult)
                    for pt, ps in ptiles:
                        han = hAnp.tile([128, D], F32R, tag="hAn")
                        nc.vector.tensor_scalar(
                            out=han[:], in0=ps[:], scalar1=scaleA[:, pt:pt + 1],
                            scalar2=None, op0=OP.mult)
                        pT = psTA.tile([128, 256], F32R, tag="psTA")
                        for c in range(2):
                            nc.tensor.transpose(pT[:, c * 128:(c + 1) * 128],
                                                han[:, c * 128:(c + 1) * 128], identr[:])
                        for c in range(2):
                            nc.scalar.copy(hAt[c][:, pt * 128:(pt + 1) * 128],
                                           pT[:, c * 128:(c + 1) * 128])

            # =========== Phase B: load hB, normalize+mask, transpose ===========
            ssB = stp.tile([128, NPT_B], F32)
            scaleB = stp.tile([128, NPT_B], F32)
            with tc.tile_pool(name="hbst", bufs=4) as hbst, \
                 tc.tile_pool(name="scrB", bufs=2) as scrB, \
                 tc.tile_pool(name="hBn", bufs=6) as hBnp, \
                 tc.tile_pool(name="psTB", bufs=2, space="PSUM") as psTB:
                hbtiles = []
                for bt in range(NPT_B):
                    s = hbst.tile([128, D], F32, tag="hbst")
                    nc.sync.dma_start(s[:], hb[bt * 128:(bt + 1) * 128, :])
                    scr = scrB.tile([128, D], F32, tag="scrB")
                    nc.scalar.activation(scr[:], s[:], ACTF.Square,
                                         accum_out=ssB[:, bt:bt + 1])
                    hbtiles.append(s)
                    if len(hbtiles) == 4 or bt == NPT_B - 1:
                        base = bt + 1 - len(hbtiles)
                        sl = slice(base, bt + 1)
                        nc.scalar.activation(scaleB[:, sl], ssB[:, sl], ACTF.Sqrt)
                        nc.vector.reciprocal(scaleB[:, sl], scaleB[:, sl])
                        nc.vector.tensor_tensor(scaleB[:, sl], scaleB[:, sl],
                                                wbt[:, sl], op=OP.mult)
                        for j, s2 in enumerate(hbtiles):
                            bt2 = base + j
                            hbn = hBnp.tile([128, D], F32R, tag="hBn")
                            nc.scalar.activation(hbn[:], s2[:], ACTF.Copy,
                                                 scale=scaleB[:, bt2:bt2 + 1])
                            for c in range(2):
                                pT = psTB.tile([128, 128], F32R, tag=f"psTB{c}")
                                nc.tensor.transpose(pT[:], hbn[:, c * 128:(c + 1) * 128],
                                                    identr[:])
                                nc.scalar.copy(hBt[c][:, bt2 * 128:(bt2 + 1) * 128], pT[:])
                        hbtiles = []

            # =========== Phase C: einsum + row/col maxes from PSUM ===========
            sAst = [stp.tile([128, 32], F32, tag=f"sAst{a}", name=f"sAst{a}") for a in range(NPT_A)]
            cBst = [stp.tile([128, 256], F32, tag=f"cBst{a}", name=f"cBst{a}") for a in range(NPT_A)]
            with tc.tile_pool(name="psC", bufs=2, space="PSUM") as psC:
                for at in range(NPT_A):
                    for g in range(NGRP):
                        ps = psC.tile([128, GRP], F32, tag="psC")
                        for k in range(2):
                            for nb in range(4):
                                nc.tensor.matmul(
                                    ps[:, nb * 512:(nb + 1) * 512],
                                    hAt[k][:, at * 128:(at + 1) * 128],
                                    hBt[k][:, g * GRP + nb * 512: g * GRP + (nb + 1) * 512],
                                    start=(k == 0), stop=(k == 1))
                        nc.vector.reduce_max(
                            sAst[at][:, g * 8:(g + 1) * 8],
                            ps[:].rearrange("p (s n) -> p s n", s=8),
                            axis=AX.X)
                        nc.vector.tensor_reduce(
                            cBst[at][:, g * 64:(g + 1) * 64],
                            ps[:].rearrange("p (j n) -> p j n", j=64),
                            axis=AX.X, op=OP.max, apply_transpose=True)

            # =========== Phase D: merges + masked-mean numerators ===========
            ostage = stp.tile([32, 2 * BA_PER], F32)
            with tc.tile_pool(name="dpool", bufs=1) as dp, \
                 tc.tile_pool(name="psD", bufs=1, space="PSUM") as psD:
                # sA: weight rows by wa, then column-sum via ones-matmul
                pa = []
                for s in range(BA_PER):
                    pd = psD.tile([32, 1], F32, tag=f"psDa{s}")
                    for half in range(2):
                        at = 2 * s + half
                        saw = dp.tile([128, 32], F32, tag=f"sAw{at}", name=f"sAw{at}")
                        nc.vector.tensor_scalar(
                            out=saw[:], in0=sAst[at][:], scalar1=wat[:, at:at + 1],
                            scalar2=None, op0=OP.mult)
                        nc.tensor.matmul(pd[:], saw[:], ones_f[:],
                                         start=(half == 0), stop=(half == 1))
                    pa.append(pd)
                # sB: shuffle 32-blocks onto partitions 0-31, fold, weight, sum
                cb2 = [dp.tile([32, 1024], F32, tag=f"cb2{at}", name=f"cb2{at}") for at in range(NPT_A)]
                f2 = [dp.tile([32, 256], F32, tag=f"f2{at}", name=f"f2{at}") for at in range(NPT_A)]
                for at in range(NPT_A):
                    for i in range(4):
                        nc.sync.dma_start(cb2[at][:, i * 256:(i + 1) * 256],
                                          cBst[at][32 * i:32 * i + 32, :])
                    f1 = dp.tile([32, 512], F32, tag="f1")
                    nc.vector.tensor_tensor(f1[:], cb2[at][:, 0:512],
                                            cb2[at][:, 512:1024], op=OP.max)
                    nc.vector.tensor_tensor(f2[at][:], f1[:, 0:256], f1[:, 256:512],
                                            op=OP.max)
                pb = []
                for s in range(BA_PER):
                    fm = dp.tile([32, 256], F32, tag="fm")
                    nc.vector.tensor_tensor(fm[:], f2[2 * s][:], f2[2 * s + 1][:], op=OP.max)
                    nc.vector.tensor_tensor(fm[:], fm[:], wbst[:], op=OP.mult)
                    sbg = dp.tile([32, 32], F32, tag="sbg")
                    nc.vector.reduce_sum(sbg[:], fm[:].rearrange("p (b n) -> p b n", b=32),
                                         axis=AX.X)
                    pd = psD.tile([32, 1], F32, tag=f"psDb{s}")
                    nc.tensor.matmul(pd[:], sbg[:], ones_f[0:32, :], start=True, stop=True)
                    pb.append(pd)
                for s in range(BA_PER):
                    nc.scalar.copy(ostage[:, s:s + 1], pa[s][:])
                    nc.scalar.copy(ostage[:, BA_PER + s:BA_PER + s + 1], pb[s][:])
            nc.sync.dma_start(osums[:], ostage[:])

    nc.compile()
    return nc


def _get_nc():
    global _NC
    if _NC is None:
        _NC = build_program()
    return _NC


def kernel(hA_raw, hB, W, b, temperature, maskA, maskB):
    global LAST_RESULTS
    hA_raw = np.ascontiguousarray(np.asarray(hA_raw, dtype=np.float32))
    hB = np.ascontiguousarray(np.asarray(hB, dtype=np.float32))
    W = np.asarray(W, dtype=np.float32)
    b = np.asarray(b, dtype=np.float32)
    temp = float(np.asarray(temperature))
    mA = np.asarray(maskA).astype(bool)
    mB = np.asarray(maskB).astype(bool)

    wt_full = np.ascontiguousarray(W.T)                       # [dA, d]
    brow = np.ascontiguousarray(b.reshape(1, D))
    hb_full = np.ascontiguousarray(hB.reshape(NTOK_B, D))
    wbf = mB.astype(np.float32).reshape(NTOK_B)               # [8192]
    wb_in = np.ascontiguousarray(wbf.reshape(NPT_B, 128).T)   # [128, 64]
    # shuffled colmax layout: token = 2048*(col//64) + 32*(col%64) + r
    col = np.arange(256)
    r = np.arange(32)
    tok_idx = 2048 * (col[None, :] // 64) + 32 * (col[None, :] % 64) + r[:, None]
    wbs_in = np.ascontiguousarray(wbf[tok_idx])               # [32, 256]

    in_maps = []
    for c in range(NCORES):
        sl = slice(c * BA_PER, (c + 1) * BA_PER)
        hat_c = np.ascontiguousarray(hA_raw[sl].reshape(NTOK_A, DA).T)  # [dA, tok]
        wa_c = np.ascontiguousarray(
            mA[sl].astype(np.float32).reshape(NPT_A, 128).T)            # [128, 8]
        in_maps.append({
            "hat": hat_c, "wt": wt_full, "brow": brow, "hb": hb_full,
            "wa": wa_c, "wb": wb_in, "wbs": wbs_in,
        })

    nc = _get_nc()
    res = run_bass_kernel_spmd(nc, in_maps, core_ids=list(range(NCORES)))
    LAST_RESULTS = res

    countA = np.maximum(mA.sum(1).astype(np.float32), 1e-6)   # [32]
    countB = np.maximum(mB.sum(1).astype(np.float32), 1e-6)   # [32]
    logits = np.zeros((B, B), np.float32)
    for c in range(NCORES):
        o = res.results[c]["osums"]                           # [32, 8]
        for a in range(BA_PER):
            ga = c * BA_PER + a
            sA = o[:, a] / countA[ga]
            sB = o[:, BA_PER + a] / countB
            logits[ga] = 0.5 * (sA + sB) / temp

    exp_logits = np.exp(logits)
    diag = np.diagonal(exp_logits)
    off = np.where(np.eye(B, dtype=bool), np.float32(0), exp_logits)
    denom_A = off.sum(axis=1)
    denom_B = off.sum(axis=0)
    loss = np.float32(0.5) * (-np.log(diag / denom_A) - np.log(diag / denom_B)).mean(
        dtype=np.float32)
    return (logits, np.float32(loss))


# revision 9
# speedup vs baseline: 1.0473x; 1.0473x over previous
"""AcaiCLIP FILIP-score + DCL-loss kernel for 8 Trainium2 NeuronCores.

Sharding: data-parallel over the bA axis (32 samples -> 4 per core). Each core
projects its hA slice, normalizes both sides (with 0/1 mask-zeroing folded into
the normalization scale), computes its [4,32,256,256] similarity slab with f32r
matmuls, reduces row-maxes (free-dim reduce) and col-maxes (DVE transpose-mode
reduce) straight from PSUM, and emits masked-mean numerators. Host assembles
the [32,32] logits and the scalar DCL loss.
"""
import os
import sys

for _p in ("/opt/trn_rl_repo", "/root/.axon_site/_ro/trn_rl_repo"):
    if os.path.isdir(_p) and _p not in sys.path:
        sys.path.insert(0, _p)

import numpy as np

import concourse.bacc as bacc
import concourse.mybir as mybir
import concourse.tile as tile
from concourse.bass_utils import run_bass_kernel_spmd
from concourse.masks import make_identity

F32 = mybir.dt.float32
F32R = mybir.dt.float32r
AX = mybir.AxisListType
OP = mybir.AluOpType
ACTF = mybir.ActivationFunctionType

B, TA, TB, DA, D = 32, 256, 256, 1024, 256
NCORES = 8
BA_PER = B // NCORES              # 4 a-samples per core
NTOK_A = BA_PER * TA              # 1024 hA tokens per core
NTOK_B = B * TB                   # 8192 hB tokens (full)
NPT_A = NTOK_A // 128             # 8 hA token tiles
NPT_B = NTOK_B // 128             # 64 hB token tiles
NKC = DA // 128                   # 8 contraction chunks for projection
GRP = 1024                        # tB columns per PSUM group (2 banks)
NGRP = NTOK_B // GRP              # 4 groups

_NC = None
LAST_RESULTS = None


def _round_to_f32r(nc, pool, src_tile, shape, name):
    t = pool.tile(shape, F32R, name=name, tag=name)
    nc.vector.tensor_copy(t[:], src_tile)
    return t


def build_program():
    nc = bacc.Bacc("TRN2")

    hat = nc.dram_tensor("hat", [NTOK_A, DA], F32, kind="ExternalInput")     # hA_raw^T slice: [dA, tok] stored [1024,1024]
    wt = nc.dram_tensor("wt", [DA, D], F32, kind="ExternalInput")            # W^T: [dA, d]
    brow = nc.dram_tensor("brow", [1, D], F32, kind="ExternalInput")
    hb = nc.dram_tensor("hb", [NTOK_B, D], F32, kind="ExternalInput")
    wa = nc.dram_tensor("wa", [128, NPT_A], F32, kind="ExternalInput")       # maskA 0/1, token t=pt*128+p -> [p, pt]
    wb = nc.dram_tensor("wb", [128, NPT_B], F32, kind="ExternalInput")       # maskB 0/1 same layout
    wbs = nc.dram_tensor("wbs", [32, 256], F32, kind="ExternalInput")        # maskB in shuffled colmax layout
    osums = nc.dram_tensor("osums", [32, 2 * BA_PER], F32, kind="ExternalOutput")

    with tile.TileContext(nc) as tc:
        with tc.tile_pool(name="const", bufs=1) as constp, \
             tc.tile_pool(name="persist", bufs=1) as pp, \
             tc.tile_pool(name="stage", bufs=1) as stp:

            # --- constants ---
            ident_f = constp.tile([128, 128], F32)
            make_identity(nc, ident_f[:])
            identr = _round_to_f32r(nc, constp, ident_f[:], [128, 128], 'identr')
            ones_f = constp.tile([128, 1], F32)
            nc.vector.memset(ones_f[:], 1.0)
            onesr = _round_to_f32r(nc, constp, ones_f[:], [128, 1], 'onesr')
            onesk_f = constp.tile([1, 128], F32)
            nc.vector.memset(onesk_f[:], 1.0)
            oneskr = _round_to_f32r(nc, constp, onesk_f[:], [1, 128], 'oneskr')

            # --- persistent data ---
            hatr = [pp.tile([128, NTOK_A], F32R, tag=f"hatr{k}", name=f"hatr{k}") for k in range(NKC)]
            wtr = [pp.tile([128, D], F32R, tag=f"wtr{k}", name=f"wtr{k}") for k in range(NKC)]
            browr = pp.tile([1, D], F32R)
            hAt = [pp.tile([128, NTOK_A], F32R, tag=f"hAt{c}", name=f"hAt{c}") for c in range(2)]
            hBt = [pp.tile([128, NTOK_B], F32R, tag=f"hBt{c}", name=f"hBt{c}") for c in range(2)]
            wat = stp.tile([128, NPT_A], F32)
            wbt = stp.tile([128, NPT_B], F32)
            wbst = stp.tile([32, 256], F32)
            nc.sync.dma_start(wat[:], wa[:])
            nc.sync.dma_start(wbt[:], wb[:])
            nc.sync.dma_start(wbst[:], wbs[:])

            # --- load + round projection operands ---
            with tc.tile_pool(name="ldst", bufs=3) as ldst:
                for k in range(NKC):
                    s = ldst.tile([128, NTOK_A], F32, tag="hatst")
                    nc.sync.dma_start(s[:], hat[k * 128:(k + 1) * 128, :])
                    nc.vector.tensor_copy(hatr[k][:], s[:])
                for k in range(NKC):
                    s = ldst.tile([128, D], F32, tag="wtst")
                    nc.sync.dma_start(s[:], wt[k * 128:(k + 1) * 128, :])
                    nc.vector.tensor_copy(wtr[k][:], s[:])
                s = ldst.tile([1, D], F32, tag="brst")
                nc.sync.dma_start(s[:], brow[:])
                nc.vector.tensor_copy(browr[:], s[:])

            # =========== Phase A: project hA, normalize+mask, transpose ===========
            ssA = stp.tile([128, NPT_A], F32)
            scaleA = stp.tile([128, NPT_A], F32)
            with tc.tile_pool(name="psA", bufs=4, space="PSUM") as psA, \
                 tc.tile_pool(name="psTA", bufs=2, space="PSUM") as psTA, \
                 tc.tile_pool(name="scrA", bufs=2) as scrA, \
                 tc.tile_pool(name="hAn", bufs=2) as hAnp:
                for wave in range(2):
                    ptiles = []
                    for pt in range(wave * 4, wave * 4 + 4):
                        ps = psA.tile([128, D], F32, tag="psA")
                        for k in range(NKC):
                            nc.tensor.matmul(
                                ps[:], hatr[k][:, pt * 128:(pt + 1) * 128], wtr[k][:],
                                start=(k == 0), stop=False)
                        nc.tensor.matmul(ps[:], oneskr[:], browr[:], start=False, stop=True)
                        scr = scrA.tile([128, D], F32, tag="scrA")
                        nc.scalar.activation(scr[:], ps[:], ACTF.Square,
                                             accum_out=ssA[:, pt:pt + 1])
                        ptiles.append((pt, ps))
                    # sqrt/recip/mask for this wave (batched over its 4 cols)
                    sl = slice(wave * 4, wave * 4 + 4)
                    nc.scalar.activation(scaleA[:, sl], ssA[:, sl], ACTF.Sqrt)
                    nc.vector.reciprocal(scaleA[:, sl], scaleA[:, sl])
                    nc.vector.tensor_tensor(scaleA[:, sl], scaleA[:, sl], wat[:, sl], op=OP.mult)
                    for pt, ps in ptiles:
                        han = hAnp.tile([128, D], F32R, tag="hAn")
                        nc.vector.tensor_scalar(
                            out=han[:], in0=ps[:], scalar1=scaleA[:, pt:pt + 1],
                            scalar2=None, op0=OP.mult)
                        pT = psTA.tile([128, 256], F32R, tag="psTA")
                        for c in range(2):
                            nc.tensor.transpose(pT[:, c * 128:(c + 1) * 128],
                                                han[:, c * 128:(c + 1) * 128], identr[:])
                        for c in range(2):
                            nc.scalar.copy(hAt[c][:, pt * 128:(pt + 1) * 128],
                                           pT[:, c * 128:(c + 1) * 128])

            # ======= Phases B+C interleaved: hB pipeline feeds the einsum =======
            ssB = stp.tile([128, NPT_B], F32)
            scaleB = stp.tile([128, NPT_B], F32)
            sAst = [stp.tile([128, 32], F32, tag=f"sAst{a}", name=f"sAst{a}") for a in range(NPT_A)]
            cBst = [stp.tile([128, 256], F32, tag=f"cBst{a}", name=f"cBst{a}") for a in range(NPT_A)]
            NB_PER_G = GRP // 128          # hB token-tiles per einsum group
            with tc.tile_pool(name="hbst", bufs=4) as hbst, \
                 tc.tile_pool(name="scrB", bufs=2) as scrB, \
                 tc.tile_pool(name="hBn", bufs=6) as hBnp, \
                 tc.tile_pool(name="psTB", bufs=1, space="PSUM") as psTB, \
                 tc.tile_pool(name="psC", bufs=3, space="PSUM") as psC:

                def emit_b_tiles(b_lo, b_hi):
                    hbtiles = []
                    for bt in range(b_lo, b_hi):
                        s = hbst.tile([128, D], F32, tag="hbst")
                        nc.sync.dma_start(s[:], hb[bt * 128:(bt + 1) * 128, :])
                        scr = scrB.tile([128, D], F32, tag="scrB")
                        nc.scalar.activation(scr[:], s[:], ACTF.Square,
                                             accum_out=ssB[:, bt:bt + 1])
                        hbtiles.append((bt, s))
                        if len(hbtiles) == 4 or bt == b_hi - 1:
                            base = hbtiles[0][0]
                            sl = slice(base, bt + 1)
                            nc.scalar.activation(scaleB[:, sl], ssB[:, sl], ACTF.Sqrt)
                            nc.vector.reciprocal(scaleB[:, sl], scaleB[:, sl])
                            nc.vector.tensor_tensor(scaleB[:, sl], scaleB[:, sl],
                                                    wbt[:, sl], op=OP.mult)
                            for bt2, s2 in hbtiles:
                                hbn = hBnp.tile([128, D], F32R, tag="hBn")
                                nc.scalar.activation(hbn[:], s2[:], ACTF.Copy,
                                                     scale=scaleB[:, bt2:bt2 + 1])
                                for c in range(2):
                                    pT = psTB.tile([128, 128], F32R, tag=f"psTB{c}")
                                    nc.tensor.transpose(pT[:], hbn[:, c * 128:(c + 1) * 128],
                                                        identr[:])
                                    nc.scalar.copy(hBt[c][:, bt2 * 128:(bt2 + 1) * 128], pT[:])
                            hbtiles = []

                def emit_c_group(at, g):
                    ps = psC.tile([128, GRP], F32, tag="psC")
                    for k in range(2):
                        for nb in range(GRP // 512):
                            nc.tensor.matmul(
                                ps[:, nb * 512:(nb + 1) * 512],
                                hAt[k][:, at * 128:(at + 1) * 128],
                                hBt[k][:, g * GRP + nb * 512: g * GRP + (nb + 1) * 512],
                                start=(k == 0), stop=(k == 1))
                    ns = GRP // 256
                    nc.vector.reduce_max(
                        sAst[at][:, g * ns:(g + 1) * ns],
                        ps[:].rearrange("p (s n) -> p s n", s=ns),
                        axis=AX.X)
                    nj = GRP // 32
                    nc.vector.tensor_reduce(
                        cBst[at][:, g * nj:(g + 1) * nj],
                        ps[:].rearrange("p (j n) -> p j n", j=nj),
                        axis=AX.X, op=OP.max, apply_transpose=True)

                for g in range(NGRP):
                    emit_b_tiles(g * NB_PER_G, (g + 1) * NB_PER_G)
                    for at in range(NPT_A):
                        emit_c_group(at, g)

            # =========== Phase D: merges + masked-mean numerators ===========
            ostage = stp.tile([32, 2 * BA_PER], F32)
            with tc.tile_pool(name="dpool", bufs=1) as dp, \
                 tc.tile_pool(name="psD", bufs=1, space="PSUM") as psD:
                # sA: weight rows by wa, then column-sum via ones-matmul
                pa = []
                for s in range(BA_PER):
                    pd = psD.tile([32, 1], F32, tag=f"psDa{s}")
                    for half in range(2):
                        at = 2 * s + half
                        saw = dp.tile([128, 32], F32, tag=f"sAw{at}", name=f"sAw{at}")
                        nc.vector.tensor_scalar(
                            out=saw[:], in0=sAst[at][:], scalar1=wat[:, at:at + 1],
                            scalar2=None, op0=OP.mult)
                        nc.tensor.matmul(pd[:], saw[:], ones_f[:],
                                         start=(half == 0), stop=(half == 1))
                    pa.append(pd)
                # sB: shuffle 32-blocks onto partitions 0-31, fold, weight, sum
                cb2 = [dp.tile([32, 1024], F32, tag=f"cb2{at}", name=f"cb2{at}") for at in range(NPT_A)]
                f2 = [dp.tile([32, 256], F32, tag=f"f2{at}", name=f"f2{at}") for at in range(NPT_A)]
                for at in range(NPT_A):
                    for i in range(4):
                        nc.sync.dma_start(cb2[at][:, i * 256:(i + 1) * 256],
                                          cBst[at][32 * i:32 * i + 32, :])
                    f1 = dp.tile([32, 512], F32, tag="f1")
                    nc.vector.tensor_tensor(f1[:], cb2[at][:, 0:512],
                                            cb2[at][:, 512:1024], op=OP.max)
                    nc.vector.tensor_tensor(f2[at][:], f1[:, 0:256], f1[:, 256:512],
                                            op=OP.max)
                pb = []
                for s in range(BA_PER):
                    fm = dp.tile([32, 256], F32, tag="fm")
                    nc.vector.tensor_tensor(fm[:], f2[2 * s][:], f2[2 * s + 1][:], op=OP.max)
                    nc.vector.tensor_tensor(fm[:], fm[:], wbst[:], op=OP.mult)
                    sbg = dp.tile([32, 32], F32, tag="sbg")
                    nc.vector.reduce_sum(sbg[:], fm[:].rearrange("p (b n) -> p b n", b=32),
                                         axis=AX.X)
                    pd = psD.tile([32, 1], F32, tag=f"psDb{s}")
                    nc.tensor.matmul(pd[:], sbg[:], ones_f[0:32, :], start=True, stop=True)
                    pb.append(pd)
                for s in range(BA_PER):
                    nc.scalar.copy(ostage[:, s:s + 1], pa[s][:])
                    nc.scalar.copy(ostage[:, BA_PER + s:BA_PER + s + 1], pb[s][:])
            nc.sync.dma_start(osums[:], ostage[:])

    nc.compile()
    return nc


def _get_nc():
    global _NC
    if _NC is None:
        _NC = build_program()
    return _NC


def kernel(hA_raw, hB, W, b, temperature, maskA, maskB):
    global LAST_RESULTS
    hA_raw = np.ascontiguousarray(np.asarray(hA_raw, dtype=np.float32))
    hB = np.ascontiguousarray(np.asarray(hB, dtype=np.float32))
    W = np.asarray(W, dtype=np.float32)
    b = np.asarray(b, dtype=np.float32)
    temp = float(np.asarray(temperature))
    mA = np.asarray(maskA).astype(bool)
    mB = np.asarray(maskB).astype(bool)

    wt_full = np.ascontiguousarray(W.T)                       # [dA, d]
    brow = np.ascontiguousarray(b.reshape(1, D))
    hb_full = np.ascontiguousarray(hB.reshape(NTOK_B, D))
    wbf = mB.astype(np.float32).reshape(NTOK_B)               # [8192]
    wb_in = np.ascontiguousarray(wbf.reshape(NPT_B, 128).T)   # [128, 64]
    # shuffled colmax layout: token = 1024*(col//32) + 32*(col%32) + r
    col = np.arange(256)
    r = np.arange(32)
    tok_idx = 1024 * (col[None, :] // 32) + 32 * (col[None, :] % 32) + r[:, None]
    wbs_in = np.ascontiguousarray(wbf[tok_idx])               # [32, 256]

    in_maps = []
    for c in range(NCORES):
        sl = slice(c * BA_PER, (c + 1) * BA_PER)
        hat_c = np.ascontiguousarray(hA_raw[sl].reshape(NTOK_A, DA).T)  # [dA, tok]
        wa_c = np.ascontiguousarray(
            mA[sl].astype(np.float32).reshape(NPT_A, 128).T)            # [128, 8]
        in_maps.append({
            "hat": hat_c, "wt": wt_full, "brow": brow, "hb": hb_full,
            "wa": wa_c, "wb": wb_in, "wbs": wbs_in,
        })

    nc = _get_nc()
    res = run_bass_kernel_spmd(nc, in_maps, core_ids=list(range(NCORES)))
    LAST_RESULTS = res

    countA = np.maximum(mA.sum(1).astype(np.float32), 1e-6)   # [32]
    countB = np.maximum(mB.sum(1).astype(np.float32), 1e-6)   # [32]
    logits = np.zeros((B, B), np.float32)
    for c in range(NCORES):
        o = res.results[c]["osums"]                           # [32, 8]
        for a in range(BA_PER):
            ga = c * BA_PER + a
            sA = o[:, a] / countA[ga]
            sB = o[:, BA_PER + a] / countB
            logits[ga] = 0.5 * (sA + sB) / temp

    exp_logits = np.exp(logits)
    diag = np.diagonal(exp_logits)
    off = np.where(np.eye(B, dtype=bool), np.float32(0), exp_logits)
    denom_A = off.sum(axis=1)
    denom_B = off.sum(axis=0)
    loss = np.float32(0.5) * (-np.log(diag / denom_A) - np.log(diag / denom_B)).mean(
        dtype=np.float32)
    return (logits, np.float32(loss))
